# revision 56
# baseline (speedup 1.0000x reference)
"""Trainium2 Bass kernel for the PCNN (piecewise-CNN) bag-classification model.

V2 design (data-parallel over sentences, 256 sentences/core):
  Host: embedding gather + channel-major fp8(e4m3) layout upload (no on-device
        gather/transpose at all).
  Device per block of 32 sentences:
    conv1d(k=3, edge-pad) as fp8 DoubleRow matmuls: channels 0..255 ride the
    pair axis (q) of 3 full-K DR matmuls (one per tap); channels 256..309 + the
    piece-0 mask row ride a row-tiled triple (3 concurrent 28-pair DR matmuls,
    one per tap, at partition bases 0/32/64).
    PCNN piecewise max-pool: mask bias +128 (fp8-exact); j0 mask rides the conv
    contraction; j1/j2 are rank-1 fp8 matmul adds into PSUM; the three phase
    maxima come from 4-unit-batched DVE reduce_max over 4 PSUM banks.
  Tail: ReLU(+bias-128), dense to 53 logits, PE transpose, bag segment-mean as
        matmul with host-built normalized selection matrix, AllReduce, softmax.
"""

import os
import sys

for _p in ("/opt/trn_rl_repo",):
    if _p not in sys.path:
        sys.path.insert(0, _p)

import numpy as np
import ml_dtypes

# ---------------- problem constants (hardcoded per spec) ----------------
N = 2048          # total sentences
L = 120           # max sentence length
LP = 122          # edge-padded length
NCORES = 8
NS = N // NCORES  # 256 sentences per core
BLK = 32          # sentences per block
NBLK = NS // BLK  # 8 blocks
SGS = 4           # sentences per matmul unit
SG_PER_BLK = BLK // SGS          # 8
SG_COLS = 512                    # padded columns per unit (4*122=488 real)
BLK_COLS = SG_PER_BLK * SG_COLS  # 4096
NF = 230
NREL = 53
NBAGS = 256
VOCAB = 100000
WD = 300
PD = 5
IN_CH = WD + 2 * PD   # 310
FCH = [(0, 128), (128, 102)]   # filter chunks
MB = 128.0            # mask bias (fp8-exact)
NPAIR_B = 27          # channel pairs in chunk B (ch 256..309)
KB = NPAIR_B + 1      # + mask row
NBR = 55              # chunk-B rows: 54 channels (256..309) + mask row

E4 = ml_dtypes.float8_e4m3fn
BF16 = ml_dtypes.bfloat16

_PROGRAM = None
LAST_RESULT = None


def _build_program():
    import concourse.bass as bass
    import concourse.mybir as mybir
    import concourse.tile as tile
    from concourse import bacc
    from concourse.masks import make_identity

    f32 = mybir.dt.float32
    bf16 = mybir.dt.bfloat16
    fp8 = mybir.dt.float8e4
    AF = mybir.ActivationFunctionType
    AX = mybir.AxisListType
    DR = mybir.MatmulPerfMode.DoubleRow

    nc = bacc.Bacc(
        "TRN2",
        target_bir_lowering=False,
        debug=False,
        num_devices=NCORES,
    )

    # ------------- external I/O -------------
    xa_d = nc.dram_tensor("xa", [NBLK, 128, 2, BLK_COLS], fp8, kind="ExternalInput").ap()
    xb_d = nc.dram_tensor("xb", [NBLK, NBR, BLK_COLS], fp8, kind="ExternalInput").ap()
    dm_d = nc.dram_tensor("dm", [NBLK, 1, 2 * BLK * L], fp8, kind="ExternalInput").ap()
    wa_d = nc.dram_tensor("wa", [128, 2, 3, 2, 128], fp8, kind="ExternalInput").ap()
    wb_d = nc.dram_tensor("wb", [NBR, 3, 2, 128], fp8, kind="ExternalInput").ap()
    snorm_d = nc.dram_tensor("snorm", [NS, NBAGS], bf16, kind="ExternalInput").ap()
    dwt_d = nc.dram_tensor("dwt", [128, 6 * NREL], bf16, kind="ExternalInput").ap()
    actb_d = nc.dram_tensor("actb", [128, 2], f32, kind="ExternalInput").ap()
    dbias_d = nc.dram_tensor("dbias", [1, NREL], bf16, kind="ExternalInput").ap()
    out_d = nc.dram_tensor("out", [NBAGS, NREL], f32, kind="ExternalOutput").ap()

    with tile.TileContext(nc) as tc:
        import contextlib

        ctx = contextlib.ExitStack()
        with ctx:
            singles = ctx.enter_context(tc.tile_pool(name="singles", bufs=1))

            # persistent tiles
            wa_sb = singles.tile([128, 2, 3, 2, 128], fp8, name="wa")
            wb_sb = singles.tile([NBR, 3, 2, 128], fp8, name="wb")
            snorm_sb = [singles.tile([128, NBAGS], bf16, name=f"sn{c}") for c in range(2)]
            dwt_sb = singles.tile([128, 6 * NREL], bf16)
            actb_sb = singles.tile([128, 2], f32)
            dbias_sb = singles.tile([1, NREL], bf16)
            ident = singles.tile([128, 128], bf16)
            ones_sb = singles.tile([1, 128], bf16)
            ones8 = singles.tile([1, 128], fp8)
            pooled = [singles.tile([128, 3, NS], f32, name=f"pool{c}") for c in range(2)]

            xa_pool = ctx.enter_context(tc.tile_pool(name="xa", bufs=2))
            xb_pool = ctx.enter_context(tc.tile_pool(name="xb", bufs=2))
            dm_pool = ctx.enter_context(tc.tile_pool(name="dm", bufs=2))
            _gu = int(os.environ.get("KERNEL_GU", "2"))
            cv_psum = ctx.enter_context(
                tc.tile_pool(name="cv", bufs=8 // _gu, space="PSUM"))

            # block-0 inputs first: the 1MB xa transfer gates the first conv
            xa0 = xa_pool.tile([128, 2, BLK_COLS], fp8, tag="xa")
            xb0 = xb_pool.tile([NBR, BLK_COLS], fp8, tag="xb")
            dm0 = dm_pool.tile([1, 2 * BLK * L], fp8, tag="dm")
            for qs in range(4):
                nc.sync.dma_start(out=xa0[32 * qs:32 * qs + 32, :, :],
                                  in_=xa_d[0, 32 * qs:32 * qs + 32, :, :])
            nc.sync.dma_start(out=xb0[:, :], in_=xb_d[0, :, :])
            nc.sync.dma_start(out=dm0[:, :], in_=dm_d[0, :, :])
            nc.sync.dma_start(out=wa_sb[:, :, :, :, :], in_=wa_d[:, :, :, :, :])
            nc.sync.dma_start(out=wb_sb[:, :, :, :], in_=wb_d[:, :, :, :])
            nc.vector.memset(ones8[:, :], 1.0)
            nc.vector.memset(pooled[0][:, :, :], 0.0)
            nc.vector.memset(pooled[1][:, :, :], 0.0)
            for c in range(2):
                nc.sync.dma_start(out=snorm_sb[c][:, :], in_=snorm_d[c * 128:(c + 1) * 128, :])
            nc.sync.dma_start(out=dwt_sb[:, :], in_=dwt_d[:, :])
            nc.sync.dma_start(out=actb_sb[:, :], in_=actb_d[:, :])
            nc.sync.dma_start(out=dbias_sb[:, :], in_=dbias_d[:, :])
            make_identity(nc, ident[:, :])
            nc.vector.memset(ones_sb[:, :], 1.0)

            UL = SGS * L   # 480 interleaved output columns per unit
            GU = int(os.environ.get("KERNEL_GU", "2"))  # units per pipeline group
            NBUF = 8 // GU
            assert cv_psum is not None

            def emit_conv(xa, xb, fc, fw, grp):
                ps = cv_psum.tile([128, GU, 512], f32, tag="cv", name=f"cv{fc}_{grp}")
                # ---- conv: 3 full DR streams per unit ----
                for tap in range(3):
                    lhsA = wa_sb[:, :, tap, fc, 0:fw]
                    for u in range(GU):
                        sg = grp * GU + u
                        base = xa[0:128, 0:2, sg * SG_COLS + SGS * tap:
                                  sg * SG_COLS + SGS * tap + 1]
                        rhs = bass.AP(
                            tensor=base.tensor, offset=base.offset,
                            ap=[base.ap[0], [BLK_COLS, 2], [1, UL]],
                        )
                        nc.tensor.matmul(
                            out=ps[0:fw, u, 0:UL],
                            lhsT=lhsA,
                            rhs=rhs,
                            start=(tap == 0),
                            stop=False,
                            perf_mode=DR,
                            skip_group_check=True,
                        )
                # ---- chunk B: taps (0,1) as a DR pair (q-stride = one token),
                #      tap 2 as a normal fp8 matmul ----
                lhsB01 = wb_sb[0:NBR, 0:2, fc, 0:fw]
                for u in range(GU):
                    sg = grp * GU + u
                    base = xb[0:NBR, sg * SG_COLS:sg * SG_COLS + 1]
                    rhs = bass.AP(
                        tensor=base.tensor, offset=base.offset,
                        ap=[base.ap[0], [SGS, 2], [1, UL]],
                    )
                    nc.tensor.matmul(
                        out=ps[0:fw, u, 0:UL],
                        lhsT=lhsB01,
                        rhs=rhs,
                        start=False,
                        stop=False,
                        perf_mode=DR,
                        skip_group_check=True,
                    )
                lhsB2 = wb_sb[0:NBR, 2, fc, 0:fw]
                for u in range(GU):
                    sg = grp * GU + u
                    nc.tensor.matmul(
                        out=ps[0:fw, u, 0:UL],
                        lhsT=lhsB2,
                        rhs=xb[0:NBR, sg * SG_COLS + 2 * SGS:
                               sg * SG_COLS + 2 * SGS + UL],
                        start=False,
                        stop=False,
                        skip_group_check=True,
                    )
                return ps

            def phase_step(st, j):
                """Emit reduce j (and the following mask-add if j<2)."""
                ps, dm, blk, fc, fw, grp = st
                s0 = blk * BLK + grp * GU * SGS
                rbase = ps[0:fw, 0:GU, 0:1]
                rin = bass.AP(
                    tensor=rbase.tensor, offset=rbase.offset,
                    ap=[rbase.ap[0], [512, GU], [1, SGS], [SGS, L]],
                )
                nc.vector.reduce_max(
                    out=pooled[fc][0:fw, j, s0:s0 + GU * SGS],
                    in_=rin,
                    axis=AX.X,
                )
                if j < 2:
                    for u in range(GU):
                        sg = grp * GU + u
                        nc.tensor.matmul(
                            out=ps[0:fw, u, 0:UL],
                            lhsT=ones8[0:1, 0:fw],
                            rhs=dm[0:1, j * BLK * L + sg * UL:
                                   j * BLK * L + (sg + 1) * UL],
                            start=False,
                            stop=(j == 1 and u == GU - 1),
                            skip_group_check=True,
                        )

            from collections import deque
            pipe = deque()  # (state, next_phase)

            def advance_pipe(skip_newest):
                if NBUF == 2:
                    # depth-2: run all phases of the previous group at once
                    n = len(pipe) - (1 if skip_newest else 0)
                    for _ in range(n):
                        st, ph = pipe.popleft()
                        for j in range(ph, 3):
                            phase_step(st, j)
                    return
                # advance in-flight groups one phase step each, youngest first
                for idx in range(len(pipe) - 1 - (1 if skip_newest else 0), -1, -1):
                    st, ph = pipe[idx]
                    if ph < 3:
                        phase_step(st, ph)
                        pipe[idx] = (st, ph + 1)
                while pipe and pipe[0][1] == 3:
                    pipe.popleft()

            for blk in range(NBLK):
                if blk == 0:
                    xa, xb, dm = xa0, xb0, dm0
                else:
                    xa = xa_pool.tile([128, 2, BLK_COLS], fp8, tag="xa")
                    xb = xb_pool.tile([NBR, BLK_COLS], fp8, tag="xb")
                    dm = dm_pool.tile([1, 2 * BLK * L], fp8, tag="dm")
                    for qs in range(4):
                        nc.sync.dma_start(out=xa[32 * qs:32 * qs + 32, :, :],
                                          in_=xa_d[blk, 32 * qs:32 * qs + 32, :, :])
                    nc.sync.dma_start(out=xb[:, :], in_=xb_d[blk, :, :])
                    nc.sync.dma_start(out=dm[:, :], in_=dm_d[blk, :, :])

                for fc, (f0, fw) in enumerate(FCH):
                    for grp in range(SG_PER_BLK // GU):
                        ps = emit_conv(xa, xb, fc, fw, grp)
                        pipe.append(((ps, dm, blk, fc, fw, grp), 0))
                        advance_pipe(skip_newest=True)
            while pipe:
                advance_pipe(skip_newest=False)

            # ---------------- tail ----------------
            pr = [singles.tile([128, 3, NS], bf16, name=f"pr{c}") for c in range(2)]
            for fc in range(2):
                nc.scalar.activation(
                    out=pr[fc][:, :, :],
                    in_=pooled[fc][:, :, :],
                    func=AF.Relu,
                    bias=actb_sb[:, fc:fc + 1],
                    scale=1.0,
                )

            # dense: logitsT [53, 256] = sum_{j,fc} dwt[(j,fc)].T @ pr
            lg_ps = cv_psum.tile([NREL, NS], f32, tag="cv", name="lgps")
            nmm = 0
            for j in range(3):
                for fc, (f0, fw) in enumerate(FCH):
                    nc.tensor.matmul(
                        out=lg_ps[:, :],
                        lhsT=dwt_sb[0:fw, (j * 2 + fc) * NREL:(j * 2 + fc + 1) * NREL],
                        rhs=pr[fc][0:fw, j, :],
                        start=(nmm == 0),
                        stop=(nmm == 5),
                    )
                    nmm += 1
            lg_sb = singles.tile([NREL, NS], bf16)
            nc.vector.tensor_copy(out=lg_sb[:, :], in_=lg_ps[:, :])

            # transpose logits -> [256 sents, 53]
            ls = [singles.tile([128, NREL], bf16, name=f"ls{c}") for c in range(2)]
            for sc in range(2):
                ltp = cv_psum.tile([128, 128], bf16, tag="cv", name="ltp")
                nc.tensor.transpose(
                    out=ltp[0:128, 0:NREL],
                    in_=lg_sb[:, sc * 128:(sc + 1) * 128],
                    identity=ident[0:NREL, 0:NREL],
                )
                nc.vector.tensor_copy(out=ls[sc][:, :], in_=ltp[0:128, 0:NREL])

            # bag aggregation: bagT [128 bags, 53] per bag-chunk (+ dense bias/8)
            cc_dram = ctx.enter_context(tc.tile_pool(name="ccd", bufs=1, space="DRAM"))
            cc_in = cc_dram.tile([NBAGS, NREL], bf16)
            cc_out = cc_dram.tile([NBAGS, NREL], bf16)
            for bc in range(2):
                bg = cv_psum.tile([128, NREL], f32, tag="cv", name="bg")
                for sc in range(2):
                    nc.tensor.matmul(
                        out=bg[:, :],
                        lhsT=snorm_sb[sc][:, bc * 128:(bc + 1) * 128],
                        rhs=ls[sc][:, :],
                        start=(sc == 0),
                        stop=False,
                    )
                nc.tensor.matmul(
                    out=bg[:, :],
                    lhsT=ones_sb[0:1, 0:128],
                    rhs=dbias_sb[0:1, :],
                    start=False,
                    stop=True,
                )
                bg_sb = singles.tile([128, NREL], bf16, name=f"bgs{bc}")
                nc.vector.tensor_copy(out=bg_sb[:, :], in_=bg[:, :])
                nc.sync.dma_start(out=cc_in[bc * 128:(bc + 1) * 128, :], in_=bg_sb[:, :])

            nc.gpsimd.collective_compute(
                "AllReduce",
                mybir.AluOpType.add,
                replica_groups=[list(range(NCORES))],
                ins=[cc_in.opt()],
                outs=[cc_out.opt()],
            )

            # softmax over the 53 relations
            for bc in range(2):
                t8 = singles.tile([128, NREL], bf16, name=f"sm8{bc}")
                nc.sync.dma_start(out=t8[:, :], in_=cc_out[bc * 128:(bc + 1) * 128, :])
                t = singles.tile([128, NREL], f32, name=f"sm{bc}")
                nc.vector.tensor_copy(out=t[:, :], in_=t8[:, :])
                nmax = singles.tile([128, 1], f32, name=f"nmax{bc}")
                nc.vector.reduce_max(out=nmax[:, :], in_=t[:, :], axis=AX.X, negate=True)
                ex = singles.tile([128, NREL], f32, name=f"ex{bc}")
                nc.scalar.activation(
                    out=ex[:, :], in_=t[:, :], func=AF.Exp, bias=nmax[:, :], scale=1.0
                )
                ssum = singles.tile([128, 1], f32, name=f"ssum{bc}")
                nc.vector.reduce_sum(out=ssum[:, :], in_=ex[:, :], axis=AX.X)
                rcp = singles.tile([128, 1], f32, name=f"rcp{bc}")
                nc.vector.reciprocal(out=rcp[:, :], in_=ssum[:, :])
                res = singles.tile([128, NREL], f32, name=f"res{bc}")
                nc.vector.tensor_scalar_mul(res[:, :], ex[:, :], rcp[:, :])
                nc.sync.dma_start(out=out_d[bc * 128:(bc + 1) * 128, :], in_=res[:, :])

    nc.compile()
    return nc


def _get_program():
    global _PROGRAM
    if _PROGRAM is None:
        _PROGRAM = _build_program()
    return _PROGRAM


def _pad_edge(a):
    return np.concatenate([a[:, :1], a, a[:, -1:]], axis=1)


def _prep_core(sentences, pos1, pos2, masks, we8, pf18, pf28):
    """Per-core input prep. Returns xa, xb, dm (all E4 uint8-backed arrays)."""
    tok = _pad_edge(sentences)        # [NS, LP]
    p1 = _pad_edge(pos1)
    p2 = _pad_edge(pos2)
    # X: [NS, LP, IN_CH] fp8 (as uint8 for speed)
    X = np.zeros((NS, LP, IN_CH), np.uint8)
    X[:, :, :WD] = we8.view(np.uint8).reshape(VOCAB, WD)[tok.reshape(-1)] \
        .reshape(NS, LP, WD)
    X[:, :, WD:WD + PD] = pf18.view(np.uint8).reshape(2 * L, PD)[p1.reshape(-1)] \
        .reshape(NS, LP, PD)
    X[:, :, WD + PD:] = pf28.view(np.uint8).reshape(2 * L, PD)[p2.reshape(-1)] \
        .reshape(NS, LP, PD)
    # channel-major interleaved stream: col = sg*512 + 4*token + sent
    buf = np.zeros((NBLK, IN_CH, SG_PER_BLK, SG_COLS), np.uint8)
    Xb = X.reshape(NBLK, SG_PER_BLK, SGS, LP, IN_CH).transpose(0, 4, 1, 3, 2)
    buf[:, :, :, :SGS * LP] = Xb.reshape(NBLK, IN_CH, SG_PER_BLK, LP * SGS)
    buf = buf.reshape(NBLK, IN_CH, BLK_COLS)

    xa = buf[:, :256].reshape(NBLK, 2, 128, BLK_COLS).transpose(0, 2, 1, 3).copy()

    xb = np.zeros((NBLK, NBR, BLK_COLS), np.uint8)
    one8 = np.float32(1.0).astype(E4).view(np.uint8)
    # mask m0 row content: value 1.0 at stream col 4*(l+1) + s (center tap)
    m0 = masks[:, 0, :]  # [NS, L] (0/1 float)
    m0row = np.zeros((NBLK, SG_PER_BLK, SG_COLS), np.uint8)
    m0v = m0row[:, :, :SGS * LP].reshape(NBLK, SG_PER_BLK, LP, SGS)
    m0v[:, :, 1:L + 1, :] = \
        (m0.reshape(NBLK, SG_PER_BLK, SGS, L).transpose(0, 1, 3, 2) > 0.5) * one8
    m0row = m0row.reshape(NBLK, BLK_COLS)
    xb[:, :NBR - 1, :] = buf[:, 256:256 + NBR - 1]
    xb[:, NBR - 1, :] = m0row

    # mask diff rows: 128*(m1-m0), 128*(m2-m1) at [blk, j, s_local*120 + l]
    # mask diff rows at interleaved cols: dm[blk, row, sg*480 + 4*l + s]
    d1 = (masks[:, 1, :] - masks[:, 0, :]) * MB
    d2 = (masks[:, 2, :] - masks[:, 1, :]) * MB
    dd = np.stack([d1, d2], axis=1).astype(np.float32) \
        .reshape(NBLK, SG_PER_BLK, SGS, 2, L).transpose(0, 3, 1, 4, 2) \
        .reshape(NBLK, 1, 2 * BLK * L)
    dm = dd.astype(E4)
    return xa.view(E4), xb.view(E4), dm


def _prep_shared(conv_w, conv_b, dense_w, dense_b, bag_ids, masks_unused=None):
    w8 = conv_w.astype(E4).astype(np.float32)  # quantize once
    wa = np.zeros((128, 2, 3, 2, 128), np.float32)
    wb = np.zeros((NBR, 3, 2, 128), np.float32)
    for fc, (f0, fw) in enumerate(FCH):
        for tap in range(3):
            for q in range(2):
                # wa[p, q, tap, fc, f] = w[f0+f, 128q+p, tap]
                wa[:, q, tap, fc, :fw] = w8[f0:f0 + fw, 128 * q:128 * (q + 1), tap].T
            # wb[c, tap, fc, f] = w[f0+f, 256+c, tap]
            wb[:NBR - 1, tap, fc, :fw] = w8[f0:f0 + fw, 256:256 + NBR - 1, tap].T
        wb[NBR - 1, 1, fc, :fw] = MB  # mask rides center tap
    wa = wa.astype(E4)
    wb = wb.astype(E4)

    actb = np.zeros((128, 2), np.float32)
    for fc, (f0, fw) in enumerate(FCH):
        actb[:fw, fc] = conv_b[f0:f0 + fw] - MB

    dwt = np.zeros((128, 6 * NREL), np.float32)
    for j in range(3):
        for fc, (f0, fw) in enumerate(FCH):
            dwt[:fw, (j * 2 + fc) * NREL:(j * 2 + fc + 1) * NREL] = \
                dense_w[:, j * NF + f0:j * NF + f0 + fw].T
    dwt = dwt.astype(BF16)
    dbias = (dense_b / NCORES).reshape(1, NREL).astype(BF16)

    counts = np.bincount(bag_ids, minlength=NBAGS).astype(np.float32)
    counts = np.maximum(counts, 1.0)
    return wa, wb, actb, dwt, dbias, counts


def kernel(**inputs):
    sentences = np.asarray(inputs["sentences"]).astype(np.int32)
    pos1 = np.asarray(inputs["pos1"]).astype(np.int32)
    pos2 = np.asarray(inputs["pos2"]).astype(np.int32)
    masks = np.asarray(inputs["masks"]).astype(np.float32)
    bag_ids = np.asarray(inputs["bag_ids"]).astype(np.int64)
    word_emb = np.asarray(inputs["word_emb"]).astype(np.float32)
    pf1_emb = np.asarray(inputs["pf1_emb"]).astype(np.float32)
    pf2_emb = np.asarray(inputs["pf2_emb"]).astype(np.float32)
    conv_w = np.asarray(inputs["conv_w"]).astype(np.float32)
    conv_b = np.asarray(inputs["conv_b"]).astype(np.float32)
    dense_w = np.asarray(inputs["dense_w"]).astype(np.float32)
    dense_b = np.asarray(inputs["dense_b"]).astype(np.float32)

    we8 = word_emb.astype(E4)
    pf18 = pf1_emb.astype(E4)
    pf28 = pf2_emb.astype(E4)

    wa, wb, actb, dwt, dbias, counts = _prep_shared(
        conv_w, conv_b, dense_w, dense_b, bag_ids)

    in_maps = []
    for r in range(NCORES):
        sl = slice(r * NS, (r + 1) * NS)
        xa, xb, dm = _prep_core(sentences[sl], pos1[sl], pos2[sl], masks[sl],
                                we8, pf18, pf28)
        bags = bag_ids[sl]
        snorm = np.zeros((NS, NBAGS), np.float32)
        snorm[np.arange(NS), bags] = 1.0 / counts[bags]
        snorm = snorm.astype(BF16)
        in_maps.append({
            "xa": xa, "xb": xb, "dm": dm,
            "wa": wa, "wb": wb,
            "snorm": snorm, "dwt": dwt, "actb": actb, "dbias": dbias,
        })

    nc = _get_program()
    from concourse.bass_utils import run_bass_kernel_spmd

    trace = bool(int(os.environ.get("KERNEL_TRACE", "0")))
    res = run_bass_kernel_spmd(
        nc, in_maps, core_ids=list(range(NCORES)), trace=trace
    )
    global LAST_RESULT
    LAST_RESULT = res
    return res.results[0]["out"].astype(np.float32)


def _selftest():
    """Numpy-emulate the device program from the prepared arrays."""
    os.environ["JAX_PLATFORMS"] = "cpu"
    sys.path.insert(0, os.path.dirname(os.path.abspath(__file__)))
    import jax
    with jax.default_device(jax.devices("cpu")[0]):
        import reference
        inputs = reference.setup_inputs()
        expected = np.asarray(reference.reference(**inputs))
        inputs = {k: np.asarray(v) for k, v in inputs.items()}

    masks = inputs["masks"].astype(np.float32)
    bag_ids = inputs["bag_ids"].astype(np.int64)
    we8 = inputs["word_emb"].astype(np.float32).astype(E4)
    pf18 = inputs["pf1_emb"].astype(np.float32).astype(E4)
    pf28 = inputs["pf2_emb"].astype(np.float32).astype(E4)
    wa, wb, actb, dwt, dbias, counts = _prep_shared(
        inputs["conv_w"].astype(np.float32), inputs["conv_b"].astype(np.float32),
        inputs["dense_w"].astype(np.float32), inputs["dense_b"].astype(np.float32),
        bag_ids)
    waf = wa.astype(np.float32)
    wbf = wb.astype(np.float32)

    out_all = np.zeros((NBAGS, NREL), np.float32)
    for r in range(NCORES):
        sl = slice(r * NS, (r + 1) * NS)
        xa, xb, dm = _prep_core(
            inputs["sentences"].astype(np.int32)[sl],
            inputs["pos1"].astype(np.int32)[sl], inputs["pos2"].astype(np.int32)[sl],
            masks[sl], we8, pf18, pf28)
        xaf = xa.astype(np.float32)
        xbf = xb.astype(np.float32)
        dmf = dm.astype(np.float32)
        pooled = np.zeros((2, 128, 3, NS), np.float32)
        for blk in range(NBLK):
            for fc, (f0, fw) in enumerate(FCH):
                for grp in range(2):
                    ps = np.zeros((fw, 4, 4, L), np.float32)
                    for u in range(4):
                        sg = grp * 4 + u
                        for tap in range(3):
                            # rhs[p, q, s, l] = xa[blk, p, q, sg*512 + 4*(l+tap) + s]
                            cols = sg * SG_COLS + SGS * tap \
                                + np.arange(SGS)[:, None] \
                                + SGS * np.arange(L)[None, :]
                            rhs = xaf[blk][:, :, cols]          # [128, 2, 4, L]
                            lhs = waf[:, :, tap, fc, :fw]       # [128, 2, fw]
                            ps[:, u] += np.einsum('pqf,pqsl->fsl', lhs, rhs)
                            rhsb = xbf[blk][:, cols]
                            lhsb = wbf[:, tap, fc, :fw]
                            ps[:, u] += np.einsum('pf,psl->fsl', lhsb, rhsb)
                    s0 = blk * BLK + grp * 16
                    for j in range(3):
                        pooled[fc, :fw, j, s0:s0 + 16] = \
                            ps.max(axis=3).transpose(0, 1, 2).reshape(fw, 16)
                        if j < 2:
                            for u in range(4):
                                sg = grp * 4 + u
                                add = dmf[blk, 0, j * BLK * L + sg * SGS * L:
                                          j * BLK * L + (sg + 1) * SGS * L] \
                                    .reshape(L, SGS).T
                                ps[:, u] += add[None, :, :]
        # tail
        pr = np.zeros((2, 128, 3, NS), np.float32)
        for fc in range(2):
            pr[fc] = np.maximum(pooled[fc] + actb[:, fc][:, None, None], 0)
        pr = pr.astype(BF16).astype(np.float32)
        dwtf = dwt.astype(np.float32)
        lg = np.zeros((NREL, NS), np.float32)
        for j in range(3):
            for fc, (f0, fw) in enumerate(FCH):
                lg += dwtf[:fw, (j * 2 + fc) * NREL:(j * 2 + fc + 1) * NREL].T @ \
                    pr[fc, :fw, j, :]
        lg = lg.astype(BF16).astype(np.float32)
        bags = bag_ids[sl]
        snorm = np.zeros((NS, NBAGS), np.float32)
        snorm[np.arange(NS), bags] = 1.0 / counts[bags]
        snorm = snorm.astype(BF16).astype(np.float32)
        out_all += snorm.T @ lg.T + dbias.astype(np.float32)

    e = np.exp(out_all - out_all.max(1, keepdims=True))
    sm = e / e.sum(1, keepdims=True)
    err = np.abs(sm - expected).max() / np.abs(expected).max()
    print("selftest rel err:", err)
    return err


if __name__ == "__main__":
    if "--selftest" in sys.argv:
        _selftest()


# revision 57
# speedup vs baseline: 1.0125x; 1.0125x over previous
"""Trainium2 Bass kernel for the PCNN (piecewise-CNN) bag-classification model.

V2 design (data-parallel over sentences, 256 sentences/core):
  Host: embedding gather + channel-major fp8(e4m3) layout upload (no on-device
        gather/transpose at all).
  Device per block of 32 sentences:
    conv1d(k=3, edge-pad) as fp8 DoubleRow matmuls: channels 0..255 ride the
    pair axis (q) of 3 full-K DR matmuls (one per tap); channels 256..309 + the
    piece-0 mask row ride a row-tiled triple (3 concurrent 28-pair DR matmuls,
    one per tap, at partition bases 0/32/64).
    PCNN piecewise max-pool: mask bias +128 (fp8-exact); j0 mask rides the conv
    contraction; j1/j2 are rank-1 fp8 matmul adds into PSUM; the three phase
    maxima come from 4-unit-batched DVE reduce_max over 4 PSUM banks.
  Tail: ReLU(+bias-128), dense to 53 logits, PE transpose, bag segment-mean as
        matmul with host-built normalized selection matrix, AllReduce, softmax.
"""

import os
import sys

for _p in ("/opt/trn_rl_repo",):
    if _p not in sys.path:
        sys.path.insert(0, _p)

import numpy as np
import ml_dtypes

# ---------------- problem constants (hardcoded per spec) ----------------
N = 2048          # total sentences
L = 120           # max sentence length
LP = 122          # edge-padded length
NCORES = 8
NS = N // NCORES  # 256 sentences per core
BLK = 32          # sentences per block
NBLK = NS // BLK  # 8 blocks
SGS = 4           # sentences per matmul unit
SG_PER_BLK = BLK // SGS          # 8
SG_COLS = 512                    # padded columns per unit (4*122=488 real)
BLK_COLS = SG_PER_BLK * SG_COLS  # 4096
NF = 230
NREL = 53
NBAGS = 256
VOCAB = 100000
WD = 300
PD = 5
IN_CH = WD + 2 * PD   # 310
FCH = [(0, 128), (128, 102)]   # filter chunks
MB = 128.0            # mask bias (fp8-exact)
NPAIR_B = 27          # channel pairs in chunk B (ch 256..309)
KB = NPAIR_B + 1      # + mask row
NBR = 55              # chunk-B rows: 54 channels (256..309) + mask row

E4 = ml_dtypes.float8_e4m3fn
BF16 = ml_dtypes.bfloat16

_PROGRAM = None
LAST_RESULT = None


def _build_program():
    import concourse.bass as bass
    import concourse.mybir as mybir
    import concourse.tile as tile
    from concourse import bacc
    from concourse.masks import make_identity

    f32 = mybir.dt.float32
    bf16 = mybir.dt.bfloat16
    fp8 = mybir.dt.float8e4
    AF = mybir.ActivationFunctionType
    AX = mybir.AxisListType
    DR = mybir.MatmulPerfMode.DoubleRow

    nc = bacc.Bacc(
        "TRN2",
        target_bir_lowering=False,
        debug=False,
        num_devices=NCORES,
    )

    # ------------- external I/O -------------
    xa_d = nc.dram_tensor("xa", [NBLK, 128, 2, BLK_COLS], fp8, kind="ExternalInput").ap()
    xb_d = nc.dram_tensor("xb", [NBLK, NBR, BLK_COLS], fp8, kind="ExternalInput").ap()
    dm_d = nc.dram_tensor("dm", [NBLK, 1, 2 * BLK * L], fp8, kind="ExternalInput").ap()
    wa_d = nc.dram_tensor("wa", [128, 2, 3, 2, 128], fp8, kind="ExternalInput").ap()
    wb_d = nc.dram_tensor("wb", [NBR, 3, 2, 128], fp8, kind="ExternalInput").ap()
    snorm_d = nc.dram_tensor("snorm", [NS, NBAGS], bf16, kind="ExternalInput").ap()
    dwt_d = nc.dram_tensor("dwt", [128, 6 * NREL], bf16, kind="ExternalInput").ap()
    actb_d = nc.dram_tensor("actb", [128, 2], f32, kind="ExternalInput").ap()
    dbias_d = nc.dram_tensor("dbias", [1, NREL], bf16, kind="ExternalInput").ap()
    out_d = nc.dram_tensor("out", [NBAGS, NREL], f32, kind="ExternalOutput").ap()

    with tile.TileContext(nc) as tc:
        import contextlib

        ctx = contextlib.ExitStack()
        with ctx:
            singles = ctx.enter_context(tc.tile_pool(name="singles", bufs=1))

            # persistent tiles
            wa_sb = singles.tile([128, 2, 3, 2, 128], fp8, name="wa")
            wb_sb = singles.tile([NBR, 3, 2, 128], fp8, name="wb")
            snorm_sb = [singles.tile([128, NBAGS], bf16, name=f"sn{c}") for c in range(2)]
            dwt_sb = singles.tile([128, 6 * NREL], bf16)
            actb_sb = singles.tile([128, 2], f32)
            dbias_sb = singles.tile([1, NREL], bf16)
            ident = singles.tile([128, 128], bf16)
            ones_sb = singles.tile([1, 128], bf16)
            ones8 = singles.tile([1, 128], fp8)
            pooled = [singles.tile([128, 3, NS], f32, name=f"pool{c}") for c in range(2)]

            xa_pool = ctx.enter_context(tc.tile_pool(name="xa", bufs=3))
            xb_pool = ctx.enter_context(tc.tile_pool(name="xb", bufs=3))
            dm_pool = ctx.enter_context(tc.tile_pool(name="dm", bufs=3))
            _gu = int(os.environ.get("KERNEL_GU", "2"))
            cv_psum = ctx.enter_context(
                tc.tile_pool(name="cv", bufs=8 // _gu, space="PSUM"))

            # block-0 inputs first: the 1MB xa transfer gates the first conv
            xa0 = xa_pool.tile([128, 2, BLK_COLS], fp8, tag="xa")
            xb0 = xb_pool.tile([NBR, BLK_COLS], fp8, tag="xb")
            dm0 = dm_pool.tile([1, 2 * BLK * L], fp8, tag="dm")
            for qs in range(4):
                nc.sync.dma_start(out=xa0[32 * qs:32 * qs + 32, :, :],
                                  in_=xa_d[0, 32 * qs:32 * qs + 32, :, :])
            nc.sync.dma_start(out=xb0[:, :], in_=xb_d[0, :, :])
            nc.sync.dma_start(out=dm0[:, :], in_=dm_d[0, :, :])
            nc.sync.dma_start(out=wa_sb[:, :, :, :, :], in_=wa_d[:, :, :, :, :])
            nc.sync.dma_start(out=wb_sb[:, :, :, :], in_=wb_d[:, :, :, :])
            nc.vector.memset(ones8[:, :], 1.0)
            nc.vector.memset(pooled[0][:, :, :], 0.0)
            nc.vector.memset(pooled[1][:, :, :], 0.0)
            for c in range(2):
                nc.sync.dma_start(out=snorm_sb[c][:, :], in_=snorm_d[c * 128:(c + 1) * 128, :])
            nc.sync.dma_start(out=dwt_sb[:, :], in_=dwt_d[:, :])
            nc.sync.dma_start(out=actb_sb[:, :], in_=actb_d[:, :])
            nc.sync.dma_start(out=dbias_sb[:, :], in_=dbias_d[:, :])
            make_identity(nc, ident[:, :])
            nc.vector.memset(ones_sb[:, :], 1.0)

            UL = SGS * L   # 480 interleaved output columns per unit
            GU = int(os.environ.get("KERNEL_GU", "2"))  # units per pipeline group
            NBUF = 8 // GU
            assert cv_psum is not None

            def emit_conv(xa, xb, fc, fw, grp):
                ps = cv_psum.tile([128, GU, 512], f32, tag="cv", name=f"cv{fc}_{grp}")
                # ---- conv: 3 full DR streams per unit ----
                for tap in range(3):
                    lhsA = wa_sb[:, :, tap, fc, 0:fw]
                    for u in range(GU):
                        sg = grp * GU + u
                        base = xa[0:128, 0:2, sg * SG_COLS + SGS * tap:
                                  sg * SG_COLS + SGS * tap + 1]
                        rhs = bass.AP(
                            tensor=base.tensor, offset=base.offset,
                            ap=[base.ap[0], [BLK_COLS, 2], [1, UL]],
                        )
                        nc.tensor.matmul(
                            out=ps[0:fw, u, 0:UL],
                            lhsT=lhsA,
                            rhs=rhs,
                            start=(tap == 0),
                            stop=False,
                            perf_mode=DR,
                            skip_group_check=True,
                        )
                # ---- chunk B: taps (0,1) as a DR pair (q-stride = one token),
                #      tap 2 as a normal fp8 matmul ----
                lhsB01 = wb_sb[0:NBR, 0:2, fc, 0:fw]
                for u in range(GU):
                    sg = grp * GU + u
                    base = xb[0:NBR, sg * SG_COLS:sg * SG_COLS + 1]
                    rhs = bass.AP(
                        tensor=base.tensor, offset=base.offset,
                        ap=[base.ap[0], [SGS, 2], [1, UL]],
                    )
                    nc.tensor.matmul(
                        out=ps[0:fw, u, 0:UL],
                        lhsT=lhsB01,
                        rhs=rhs,
                        start=False,
                        stop=False,
                        perf_mode=DR,
                        skip_group_check=True,
                    )
                lhsB2 = wb_sb[0:NBR, 2, fc, 0:fw]
                for u in range(GU):
                    sg = grp * GU + u
                    nc.tensor.matmul(
                        out=ps[0:fw, u, 0:UL],
                        lhsT=lhsB2,
                        rhs=xb[0:NBR, sg * SG_COLS + 2 * SGS:
                               sg * SG_COLS + 2 * SGS + UL],
                        start=False,
                        stop=False,
                        skip_group_check=True,
                    )
                return ps

            def phase_step(st, j):
                """Emit reduce j (and the following mask-add if j<2)."""
                ps, dm, blk, fc, fw, grp = st
                s0 = blk * BLK + grp * GU * SGS
                rbase = ps[0:fw, 0:GU, 0:1]
                rin = bass.AP(
                    tensor=rbase.tensor, offset=rbase.offset,
                    ap=[rbase.ap[0], [512, GU], [1, SGS], [SGS, L]],
                )
                nc.vector.reduce_max(
                    out=pooled[fc][0:fw, j, s0:s0 + GU * SGS],
                    in_=rin,
                    axis=AX.X,
                )
                if j < 2:
                    for u in range(GU):
                        sg = grp * GU + u
                        nc.tensor.matmul(
                            out=ps[0:fw, u, 0:UL],
                            lhsT=ones8[0:1, 0:fw],
                            rhs=dm[0:1, j * BLK * L + sg * UL:
                                   j * BLK * L + (sg + 1) * UL],
                            start=False,
                            stop=(j == 1 and u == GU - 1),
                            skip_group_check=True,
                        )

            from collections import deque
            pipe = deque()  # (state, next_phase)

            def advance_pipe(skip_newest):
                if NBUF == 2:
                    # depth-2: run all phases of the previous group at once
                    n = len(pipe) - (1 if skip_newest else 0)
                    for _ in range(n):
                        st, ph = pipe.popleft()
                        for j in range(ph, 3):
                            phase_step(st, j)
                    return
                # advance in-flight groups one phase step each, youngest first
                for idx in range(len(pipe) - 1 - (1 if skip_newest else 0), -1, -1):
                    st, ph = pipe[idx]
                    if ph < 3:
                        phase_step(st, ph)
                        pipe[idx] = (st, ph + 1)
                while pipe and pipe[0][1] == 3:
                    pipe.popleft()

            for blk in range(NBLK):
                if blk == 0:
                    xa, xb, dm = xa0, xb0, dm0
                else:
                    xa = xa_pool.tile([128, 2, BLK_COLS], fp8, tag="xa")
                    xb = xb_pool.tile([NBR, BLK_COLS], fp8, tag="xb")
                    dm = dm_pool.tile([1, 2 * BLK * L], fp8, tag="dm")
                    for qs in range(4):
                        nc.sync.dma_start(out=xa[32 * qs:32 * qs + 32, :, :],
                                          in_=xa_d[blk, 32 * qs:32 * qs + 32, :, :])
                    nc.sync.dma_start(out=xb[:, :], in_=xb_d[blk, :, :])
                    nc.sync.dma_start(out=dm[:, :], in_=dm_d[blk, :, :])

                for fc, (f0, fw) in enumerate(FCH):
                    for grp in range(SG_PER_BLK // GU):
                        ps = emit_conv(xa, xb, fc, fw, grp)
                        pipe.append(((ps, dm, blk, fc, fw, grp), 0))
                        advance_pipe(skip_newest=True)
            while pipe:
                advance_pipe(skip_newest=False)

            # ---------------- tail ----------------
            pr = [singles.tile([128, 3, NS], bf16, name=f"pr{c}") for c in range(2)]
            for fc in range(2):
                nc.scalar.activation(
                    out=pr[fc][:, :, :],
                    in_=pooled[fc][:, :, :],
                    func=AF.Relu,
                    bias=actb_sb[:, fc:fc + 1],
                    scale=1.0,
                )

            # dense: logitsT [53, 256] = sum_{j,fc} dwt[(j,fc)].T @ pr
            lg_ps = cv_psum.tile([NREL, NS], f32, tag="cv", name="lgps")
            nmm = 0
            for j in range(3):
                for fc, (f0, fw) in enumerate(FCH):
                    nc.tensor.matmul(
                        out=lg_ps[:, :],
                        lhsT=dwt_sb[0:fw, (j * 2 + fc) * NREL:(j * 2 + fc + 1) * NREL],
                        rhs=pr[fc][0:fw, j, :],
                        start=(nmm == 0),
                        stop=(nmm == 5),
                    )
                    nmm += 1
            lg_sb = singles.tile([NREL, NS], bf16)
            nc.vector.tensor_copy(out=lg_sb[:, :], in_=lg_ps[:, :])

            # transpose logits -> [256 sents, 53]
            ls = [singles.tile([128, NREL], bf16, name=f"ls{c}") for c in range(2)]
            for sc in range(2):
                ltp = cv_psum.tile([128, 128], bf16, tag="cv", name="ltp")
                nc.tensor.transpose(
                    out=ltp[0:128, 0:NREL],
                    in_=lg_sb[:, sc * 128:(sc + 1) * 128],
                    identity=ident[0:NREL, 0:NREL],
                )
                nc.vector.tensor_copy(out=ls[sc][:, :], in_=ltp[0:128, 0:NREL])

            # bag aggregation: bagT [128 bags, 53] per bag-chunk (+ dense bias/8)
            cc_dram = ctx.enter_context(tc.tile_pool(name="ccd", bufs=1, space="DRAM"))
            cc_in = cc_dram.tile([NBAGS, NREL], bf16)
            cc_out = cc_dram.tile([NBAGS, NREL], bf16)
            for bc in range(2):
                bg = cv_psum.tile([128, NREL], f32, tag="cv", name="bg")
                for sc in range(2):
                    nc.tensor.matmul(
                        out=bg[:, :],
                        lhsT=snorm_sb[sc][:, bc * 128:(bc + 1) * 128],
                        rhs=ls[sc][:, :],
                        start=(sc == 0),
                        stop=False,
                    )
                nc.tensor.matmul(
                    out=bg[:, :],
                    lhsT=ones_sb[0:1, 0:128],
                    rhs=dbias_sb[0:1, :],
                    start=False,
                    stop=True,
                )
                bg_sb = singles.tile([128, NREL], bf16, name=f"bgs{bc}")
                nc.vector.tensor_copy(out=bg_sb[:, :], in_=bg[:, :])
                nc.sync.dma_start(out=cc_in[bc * 128:(bc + 1) * 128, :], in_=bg_sb[:, :])

            nc.gpsimd.collective_compute(
                "AllReduce",
                mybir.AluOpType.add,
                replica_groups=[list(range(NCORES))],
                ins=[cc_in.opt()],
                outs=[cc_out.opt()],
            )

            # softmax over the 53 relations
            for bc in range(2):
                t8 = singles.tile([128, NREL], bf16, name=f"sm8{bc}")
                nc.sync.dma_start(out=t8[:, :], in_=cc_out[bc * 128:(bc + 1) * 128, :])
                t = singles.tile([128, NREL], f32, name=f"sm{bc}")
                nc.vector.tensor_copy(out=t[:, :], in_=t8[:, :])
                nmax = singles.tile([128, 1], f32, name=f"nmax{bc}")
                nc.vector.reduce_max(out=nmax[:, :], in_=t[:, :], axis=AX.X, negate=True)
                ex = singles.tile([128, NREL], f32, name=f"ex{bc}")
                nc.scalar.activation(
                    out=ex[:, :], in_=t[:, :], func=AF.Exp, bias=nmax[:, :], scale=1.0
                )
                ssum = singles.tile([128, 1], f32, name=f"ssum{bc}")
                nc.vector.reduce_sum(out=ssum[:, :], in_=ex[:, :], axis=AX.X)
                rcp = singles.tile([128, 1], f32, name=f"rcp{bc}")
                nc.vector.reciprocal(out=rcp[:, :], in_=ssum[:, :])
                res = singles.tile([128, NREL], f32, name=f"res{bc}")
                nc.vector.tensor_scalar_mul(res[:, :], ex[:, :], rcp[:, :])
                nc.sync.dma_start(out=out_d[bc * 128:(bc + 1) * 128, :], in_=res[:, :])

    nc.compile()
    return nc


def _get_program():
    global _PROGRAM
    if _PROGRAM is None:
        _PROGRAM = _build_program()
    return _PROGRAM


def _pad_edge(a):
    return np.concatenate([a[:, :1], a, a[:, -1:]], axis=1)


def _prep_core(sentences, pos1, pos2, masks, we8, pf18, pf28):
    """Per-core input prep. Returns xa, xb, dm (all E4 uint8-backed arrays)."""
    tok = _pad_edge(sentences)        # [NS, LP]
    p1 = _pad_edge(pos1)
    p2 = _pad_edge(pos2)
    # X: [NS, LP, IN_CH] fp8 (as uint8 for speed)
    X = np.zeros((NS, LP, IN_CH), np.uint8)
    X[:, :, :WD] = we8.view(np.uint8).reshape(VOCAB, WD)[tok.reshape(-1)] \
        .reshape(NS, LP, WD)
    X[:, :, WD:WD + PD] = pf18.view(np.uint8).reshape(2 * L, PD)[p1.reshape(-1)] \
        .reshape(NS, LP, PD)
    X[:, :, WD + PD:] = pf28.view(np.uint8).reshape(2 * L, PD)[p2.reshape(-1)] \
        .reshape(NS, LP, PD)
    # channel-major interleaved stream: col = sg*512 + 4*token + sent
    buf = np.zeros((NBLK, IN_CH, SG_PER_BLK, SG_COLS), np.uint8)
    Xb = X.reshape(NBLK, SG_PER_BLK, SGS, LP, IN_CH).transpose(0, 4, 1, 3, 2)
    buf[:, :, :, :SGS * LP] = Xb.reshape(NBLK, IN_CH, SG_PER_BLK, LP * SGS)
    buf = buf.reshape(NBLK, IN_CH, BLK_COLS)

    xa = buf[:, :256].reshape(NBLK, 2, 128, BLK_COLS).transpose(0, 2, 1, 3).copy()

    xb = np.zeros((NBLK, NBR, BLK_COLS), np.uint8)
    one8 = np.float32(1.0).astype(E4).view(np.uint8)
    # mask m0 row content: value 1.0 at stream col 4*(l+1) + s (center tap)
    m0 = masks[:, 0, :]  # [NS, L] (0/1 float)
    m0row = np.zeros((NBLK, SG_PER_BLK, SG_COLS), np.uint8)
    m0v = m0row[:, :, :SGS * LP].reshape(NBLK, SG_PER_BLK, LP, SGS)
    m0v[:, :, 1:L + 1, :] = \
        (m0.reshape(NBLK, SG_PER_BLK, SGS, L).transpose(0, 1, 3, 2) > 0.5) * one8
    m0row = m0row.reshape(NBLK, BLK_COLS)
    xb[:, :NBR - 1, :] = buf[:, 256:256 + NBR - 1]
    xb[:, NBR - 1, :] = m0row

    # mask diff rows: 128*(m1-m0), 128*(m2-m1) at [blk, j, s_local*120 + l]
    # mask diff rows at interleaved cols: dm[blk, row, sg*480 + 4*l + s]
    d1 = (masks[:, 1, :] - masks[:, 0, :]) * MB
    d2 = (masks[:, 2, :] - masks[:, 1, :]) * MB
    dd = np.stack([d1, d2], axis=1).astype(np.float32) \
        .reshape(NBLK, SG_PER_BLK, SGS, 2, L).transpose(0, 3, 1, 4, 2) \
        .reshape(NBLK, 1, 2 * BLK * L)
    dm = dd.astype(E4)
    return xa.view(E4), xb.view(E4), dm


def _prep_shared(conv_w, conv_b, dense_w, dense_b, bag_ids, masks_unused=None):
    w8 = conv_w.astype(E4).astype(np.float32)  # quantize once
    wa = np.zeros((128, 2, 3, 2, 128), np.float32)
    wb = np.zeros((NBR, 3, 2, 128), np.float32)
    for fc, (f0, fw) in enumerate(FCH):
        for tap in range(3):
            for q in range(2):
                # wa[p, q, tap, fc, f] = w[f0+f, 128q+p, tap]
                wa[:, q, tap, fc, :fw] = w8[f0:f0 + fw, 128 * q:128 * (q + 1), tap].T
            # wb[c, tap, fc, f] = w[f0+f, 256+c, tap]
            wb[:NBR - 1, tap, fc, :fw] = w8[f0:f0 + fw, 256:256 + NBR - 1, tap].T
        wb[NBR - 1, 1, fc, :fw] = MB  # mask rides center tap
    wa = wa.astype(E4)
    wb = wb.astype(E4)

    actb = np.zeros((128, 2), np.float32)
    for fc, (f0, fw) in enumerate(FCH):
        actb[:fw, fc] = conv_b[f0:f0 + fw] - MB

    dwt = np.zeros((128, 6 * NREL), np.float32)
    for j in range(3):
        for fc, (f0, fw) in enumerate(FCH):
            dwt[:fw, (j * 2 + fc) * NREL:(j * 2 + fc + 1) * NREL] = \
                dense_w[:, j * NF + f0:j * NF + f0 + fw].T
    dwt = dwt.astype(BF16)
    dbias = (dense_b / NCORES).reshape(1, NREL).astype(BF16)

    counts = np.bincount(bag_ids, minlength=NBAGS).astype(np.float32)
    counts = np.maximum(counts, 1.0)
    return wa, wb, actb, dwt, dbias, counts


def kernel(**inputs):
    sentences = np.asarray(inputs["sentences"]).astype(np.int32)
    pos1 = np.asarray(inputs["pos1"]).astype(np.int32)
    pos2 = np.asarray(inputs["pos2"]).astype(np.int32)
    masks = np.asarray(inputs["masks"]).astype(np.float32)
    bag_ids = np.asarray(inputs["bag_ids"]).astype(np.int64)
    word_emb = np.asarray(inputs["word_emb"]).astype(np.float32)
    pf1_emb = np.asarray(inputs["pf1_emb"]).astype(np.float32)
    pf2_emb = np.asarray(inputs["pf2_emb"]).astype(np.float32)
    conv_w = np.asarray(inputs["conv_w"]).astype(np.float32)
    conv_b = np.asarray(inputs["conv_b"]).astype(np.float32)
    dense_w = np.asarray(inputs["dense_w"]).astype(np.float32)
    dense_b = np.asarray(inputs["dense_b"]).astype(np.float32)

    we8 = word_emb.astype(E4)
    pf18 = pf1_emb.astype(E4)
    pf28 = pf2_emb.astype(E4)

    wa, wb, actb, dwt, dbias, counts = _prep_shared(
        conv_w, conv_b, dense_w, dense_b, bag_ids)

    in_maps = []
    for r in range(NCORES):
        sl = slice(r * NS, (r + 1) * NS)
        xa, xb, dm = _prep_core(sentences[sl], pos1[sl], pos2[sl], masks[sl],
                                we8, pf18, pf28)
        bags = bag_ids[sl]
        snorm = np.zeros((NS, NBAGS), np.float32)
        snorm[np.arange(NS), bags] = 1.0 / counts[bags]
        snorm = snorm.astype(BF16)
        in_maps.append({
            "xa": xa, "xb": xb, "dm": dm,
            "wa": wa, "wb": wb,
            "snorm": snorm, "dwt": dwt, "actb": actb, "dbias": dbias,
        })

    nc = _get_program()
    from concourse.bass_utils import run_bass_kernel_spmd

    trace = bool(int(os.environ.get("KERNEL_TRACE", "0")))
    res = run_bass_kernel_spmd(
        nc, in_maps, core_ids=list(range(NCORES)), trace=trace
    )
    global LAST_RESULT
    LAST_RESULT = res
    return res.results[0]["out"].astype(np.float32)


def _selftest():
    """Numpy-emulate the device program from the prepared arrays."""
    os.environ["JAX_PLATFORMS"] = "cpu"
    sys.path.insert(0, os.path.dirname(os.path.abspath(__file__)))
    import jax
    with jax.default_device(jax.devices("cpu")[0]):
        import reference
        inputs = reference.setup_inputs()
        expected = np.asarray(reference.reference(**inputs))
        inputs = {k: np.asarray(v) for k, v in inputs.items()}

    masks = inputs["masks"].astype(np.float32)
    bag_ids = inputs["bag_ids"].astype(np.int64)
    we8 = inputs["word_emb"].astype(np.float32).astype(E4)
    pf18 = inputs["pf1_emb"].astype(np.float32).astype(E4)
    pf28 = inputs["pf2_emb"].astype(np.float32).astype(E4)
    wa, wb, actb, dwt, dbias, counts = _prep_shared(
        inputs["conv_w"].astype(np.float32), inputs["conv_b"].astype(np.float32),
        inputs["dense_w"].astype(np.float32), inputs["dense_b"].astype(np.float32),
        bag_ids)
    waf = wa.astype(np.float32)
    wbf = wb.astype(np.float32)

    out_all = np.zeros((NBAGS, NREL), np.float32)
    for r in range(NCORES):
        sl = slice(r * NS, (r + 1) * NS)
        xa, xb, dm = _prep_core(
            inputs["sentences"].astype(np.int32)[sl],
            inputs["pos1"].astype(np.int32)[sl], inputs["pos2"].astype(np.int32)[sl],
            masks[sl], we8, pf18, pf28)
        xaf = xa.astype(np.float32)
        xbf = xb.astype(np.float32)
        dmf = dm.astype(np.float32)
        pooled = np.zeros((2, 128, 3, NS), np.float32)
        for blk in range(NBLK):
            for fc, (f0, fw) in enumerate(FCH):
                for grp in range(2):
                    ps = np.zeros((fw, 4, 4, L), np.float32)
                    for u in range(4):
                        sg = grp * 4 + u
                        for tap in range(3):
                            # rhs[p, q, s, l] = xa[blk, p, q, sg*512 + 4*(l+tap) + s]
                            cols = sg * SG_COLS + SGS * tap \
                                + np.arange(SGS)[:, None] \
                                + SGS * np.arange(L)[None, :]
                            rhs = xaf[blk][:, :, cols]          # [128, 2, 4, L]
                            lhs = waf[:, :, tap, fc, :fw]       # [128, 2, fw]
                            ps[:, u] += np.einsum('pqf,pqsl->fsl', lhs, rhs)
                            rhsb = xbf[blk][:, cols]
                            lhsb = wbf[:, tap, fc, :fw]
                            ps[:, u] += np.einsum('pf,psl->fsl', lhsb, rhsb)
                    s0 = blk * BLK + grp * 16
                    for j in range(3):
                        pooled[fc, :fw, j, s0:s0 + 16] = \
                            ps.max(axis=3).transpose(0, 1, 2).reshape(fw, 16)
                        if j < 2:
                            for u in range(4):
                                sg = grp * 4 + u
                                add = dmf[blk, 0, j * BLK * L + sg * SGS * L:
                                          j * BLK * L + (sg + 1) * SGS * L] \
                                    .reshape(L, SGS).T
                                ps[:, u] += add[None, :, :]
        # tail
        pr = np.zeros((2, 128, 3, NS), np.float32)
        for fc in range(2):
            pr[fc] = np.maximum(pooled[fc] + actb[:, fc][:, None, None], 0)
        pr = pr.astype(BF16).astype(np.float32)
        dwtf = dwt.astype(np.float32)
        lg = np.zeros((NREL, NS), np.float32)
        for j in range(3):
            for fc, (f0, fw) in enumerate(FCH):
                lg += dwtf[:fw, (j * 2 + fc) * NREL:(j * 2 + fc + 1) * NREL].T @ \
                    pr[fc, :fw, j, :]
        lg = lg.astype(BF16).astype(np.float32)
        bags = bag_ids[sl]
        snorm = np.zeros((NS, NBAGS), np.float32)
        snorm[np.arange(NS), bags] = 1.0 / counts[bags]
        snorm = snorm.astype(BF16).astype(np.float32)
        out_all += snorm.T @ lg.T + dbias.astype(np.float32)

    e = np.exp(out_all - out_all.max(1, keepdims=True))
    sm = e / e.sum(1, keepdims=True)
    err = np.abs(sm - expected).max() / np.abs(expected).max()
    print("selftest rel err:", err)
    return err


if __name__ == "__main__":
    if "--selftest" in sys.argv:
        _selftest()
